# revision 32
# baseline (speedup 1.0000x reference)
"""Trainium2 Bass kernel for single-head causal attention (nn_DefaultAttention).

Reference computation (B=4, S=2048, E=1024, fp32):
    k = x @ Wk.T + bk ; q = x @ Wq.T + bq ; v = x @ Wv.T + bv
    sim[b,s,t] = k[b,s]·q[b,t] / sqrt(E), masked to t<=s
    out[b,s]   = softmax_t(sim[b,s,:]) @ v[b,:]
i.e. standard causal attention with Q-role=k, K-role=q, V-role=v.

Sharding: 8 cores = 4 batches x 2 interleaved sequence-quarter sets.
With 256-row query blocks g0..g7 per batch, core h=0 takes {g0,g2,g5,g7}
and h=1 takes {g1,g3,g4,g6} (balanced causal work: 36 quarter-tiles each).
SPMD requires an identical instruction stream on every core, so the
causal structure is made uniform via a host-side per-core column
permutation of x^T plus data-driven 0/1 masks. Score slots sigma=0..3
process T=[16,12,8,4] key-tiles against query blocks at fixed permuted
positions SRC=[1536,1024,512,0]; the permutations place each core's
blocks so every needed key precedes the slot's window:
  h=0 perm (256-blocks): [0,1,2,3,5,4,7,6]
  h=1 perm (256-blocks): [1,0,3,2,4,5,6,7]
Invalid (t>s) positions are zeroed after exp by per-core mask tensors.

All matmuls run as float32r (single-pass fp32, 1 cycle/row at N>=256).
Projections compute kT/qT in [feature, seq] layout directly (host feeds
x^T and W^T so no on-chip transposes); scores are computed transposed
(simT[t,s]) so P=exp(simT) feeds the attn@v matmul as the streaming
operand with v slices stationary; output comes out as out^T[d,s] and is
transposed back on the host. Softmax denominator = ones-vector matmul
over P, normalization by a broadcast reciprocal at the end.
"""

import numpy as np


def _ensure_concourse():
    try:
        import concourse  # noqa: F401
    except ImportError:
        import sys
        for p in ("/opt/trn_rl_repo", "/root/.axon_site/_ro/trn_rl_repo"):
            if p not in sys.path:
                sys.path.append(p)
        import concourse  # noqa: F401


E = 1024
S = 2048
B = 4
NCORES = 8
ET = E // 128    # 8 feature tiles
ST = S // 128    # 16 key tiles
SCALE = 1.0 / np.sqrt(np.float32(E))
SLOT_T = (16, 12, 8, 4)        # key-128-tiles per score slot (uniform)
SRC = (1536, 1024, 512, 0)     # permuted query-col base per slot
MBASE = (0, 16, 28, 36)        # flat mask index base per slot
SLOT_ORDER = (3, 2, 1, 0)      # processing order (smallest T first)
PERM_BLOCKS = {0: [0, 1, 2, 3, 5, 4, 7, 6], 1: [1, 0, 3, 2, 4, 5, 6, 7]}

_CACHE = {}


def _build_program():
    _ensure_concourse()
    from contextlib import ExitStack
    import concourse.tile as tile
    import concourse.bass as bass
    from concourse import bacc, mybir

    F32 = mybir.dt.float32
    F32R = mybir.dt.float32r
    ts = bass.ts
    Exp = mybir.ActivationFunctionType.Exp
    Ident = mybir.ActivationFunctionType.Identity

    nc = bacc.Bacc("TRN2", target_bir_lowering=False, debug=False)

    xT = nc.dram_tensor("xT", [E, S], F32R, kind="ExternalInput").ap()
    wkT = nc.dram_tensor("wkT", [E, E], F32R, kind="ExternalInput").ap()
    wqT = nc.dram_tensor("wqT", [E, E], F32R, kind="ExternalInput").ap()
    wvT = nc.dram_tensor("wvT", [E, E], F32R, kind="ExternalInput").ap()
    bkp = nc.dram_tensor("bkp", [128, ET], F32, kind="ExternalInput").ap()
    bqp = nc.dram_tensor("bqp", [128, ET], F32, kind="ExternalInput").ap()
    bv = nc.dram_tensor("bv", [E], F32, kind="ExternalInput").ap()
    masks = nc.dram_tensor("masks", [40, 128, 256], F32R, kind="ExternalInput").ap()
    ones_d = nc.dram_tensor("ones_d", [128, 128], F32R, kind="ExternalInput").ap()
    outT = nc.dram_tensor("outT", [E, 1024], F32, kind="ExternalOutput").ap()
    v_dram = nc.dram_tensor("v_spill", [S, E], F32R).ap()

    with tile.TileContext(nc) as tc, ExitStack() as top:
        # ---- persistent smalls -------------------------------------------
        smalls = top.enter_context(tc.tile_pool(name="smalls", bufs=1))
        ones = smalls.tile([128, 128], F32R)
        nc.sync.dma_start(out=ones, in_=ones_d)
        bk_sb = smalls.tile([128, ET], F32)
        nc.sync.dma_start(out=bk_sb, in_=bkp)
        bq_sb = smalls.tile([128, ET], F32)
        nc.sync.dma_start(out=bq_sb, in_=bqp)

        # Warm the ACT function tables (Identity/Exp/Copy) up front so the
        # LoadActFuncSet DMA doesn't queue behind the bulk loads later.
        scratch = smalls.tile([1, 8], F32)
        nc.vector.memset(scratch, 0.0)
        nc.scalar.activation(scratch, scratch, Ident, bias=0.0, scale=1.0)
        nc.scalar.activation(scratch, scratch, Exp, scale=1.0)
        nc.scalar.copy(scratch, scratch)

        # x^T resident for the three projection phases (closed after K)
        xt_ctx = tc.tile_pool(name="xt", bufs=1)
        xt_pool = xt_ctx.__enter__()
        xt = xt_pool.tile([128, ET, S], F32R)

        def load_xt(cb):
            for e in range(ET):
                nc.sync.dma_start(
                    out=xt[:, e, ts(cb, 512)],
                    in_=xT.rearrange("(e p) s -> p e s", p=128)[:, e, ts(cb, 512)],
                )

        # Weight pools opened early; close LIFO (wv after V, wq after Q,
        # wk after K). The wk/wq loads are trickled into the spill stream.
        wk_ctx = tc.tile_pool(name="wk", bufs=1)
        wk_pool = wk_ctx.__enter__()
        wk = wk_pool.tile([128, ET, E], F32R)
        wq_ctx = tc.tile_pool(name="wq", bufs=1)
        wq_pool = wq_ctx.__enter__()
        wq = wq_pool.tile([128, ET, E], F32R)

        wkq_thunks = []
        for _db in range(2):
            for _e in range(ET):
                wkq_thunks.append((wq, wqT, _e, _db))
        for _db in range(2):
            for _e in range(ET):
                wkq_thunks.append((wk, wkT, _e, _db))
        # V-phase DMA is budget-limited (loads + v spills saturate ~360GB/s
        # for the whole phase), so only wq-db0 is trickled there; the rest
        # trickles during Q, whose DMA queue is otherwise idle. wq db1 is
        # first consumed at ft=4, a third into Q; wk only at phase K.

        def emit_wkq(n):
            for _ in range(n):
                if not wkq_thunks:
                    return
                dst, src, _e, _db = wkq_thunks.pop(0)
                nc.sync.dma_start(
                    out=dst[:, _e, ts(_db, 512)],
                    in_=src.rearrange("(e p) f -> p e f", p=128)[:, _e, ts(_db, 512)],
                )

        # ---- phase V: v = x @ Wv.T + bv  (natural [t, d]), spilled to DRAM
        with tc.tile_pool(name="wv", bufs=1) as wv_pool, \
             tc.tile_pool(name="vsb", bufs=16) as vsb_pool, \
             tc.tile_pool(name="bvb", bufs=1) as bvb_pool, \
             tc.tile_pool(name="pv", bufs=6, space="PSUM") as pv_pool:
            wv = wv_pool.tile([128, ET, E], F32R)

            def load_wv(db):
                for e in range(ET):
                    nc.sync.dma_start(
                        out=wv[:, e, ts(db, 512)],
                        in_=wvT.rearrange("(e p) d -> p e d", p=128)[:, e, ts(db, 512)],
                    )
            bvb = bvb_pool.tile([128, E], F32)
            bv_bcast = bass.AP(tensor=bv.tensor, offset=bv.offset,
                               ap=[[0, 128]] + list(bv.ap))
            def load_wv_half(db, half):
                for e in range(4 * half, 4 * half + 4):
                    nc.sync.dma_start(
                        out=wv[:, e, ts(db, 512)],
                        in_=wvT.rearrange("(e p) d -> p e d", p=128)[:, e, ts(db, 512)],
                    )
            load_wv(0)
            load_xt(0)
            nc.sync.dma_start(out=bvb, in_=bv_bcast)
            load_xt(1)
            load_xt(2)
            load_wv_half(1, 0)
            load_xt(3)
            load_wv_half(1, 1)

            for db in range(2):
                for tt in range(ST):
                    pv = pv_pool.tile([128, 512], F32, tag="pv")
                    for e in range(ET):
                        nc.tensor.matmul(
                            pv, xt[:, e, ts(tt, 128)], wv[:, e, ts(db, 512)],
                            start=(e == 0), stop=(e == ET - 1),
                        )
                    vhalf = vsb_pool.tile([128, 512], F32R, tag="vsb")
                    nc.vector.tensor_add(vhalf, pv, bvb[:, ts(db, 512)])
                    nc.sync.dma_start(
                        out=v_dram[ts(tt, 128), ts(db, 512)], in_=vhalf)
                    if db == 0 and tt < 8:
                        emit_wkq(1)

        # ---- phase Q: qT = (x @ Wq.T + bq)^T  ([f, t], all 2048 t) -------
        qt_ctx = tc.tile_pool(name="qt", bufs=1, side="right")
        qt_pool = qt_ctx.__enter__()
        qt = qt_pool.tile([128, ET, S], F32R)
        with tc.tile_pool(name="pq", bufs=8, space="PSUM") as pq_pool:
            for ft in range(ET):
                for sb4 in range(4):
                    pq = pq_pool.tile([128, 512], F32, tag="pq")
                    for e in range(ET):
                        nc.tensor.matmul(
                            pq, wq[:, e, ts(ft, 128)], xt[:, e, ts(sb4, 512)],
                            start=(e == 0), stop=(e == ET - 1),
                        )
                    nc.scalar.activation(qt[:, ft, ts(sb4, 512)], pq, Ident,
                                         bias=bq_sb[:, ft:ft + 1])
                    emit_wkq(1)
        emit_wkq(99)
        wq_ctx.__exit__(None, None, None)

        # ---- phase K: kT for local queries, slot sg at cols [256*sg,..) --
        kt_ctx = tc.tile_pool(name="kt", bufs=1, side="right")
        kt_pool = kt_ctx.__enter__()
        kt = kt_pool.tile([128, ET, 1024], F32R)
        with tc.tile_pool(name="pk", bufs=6, space="PSUM") as pk_pool:
            for ft in range(ET):
                for sg in SLOT_ORDER:
                    pk = pk_pool.tile([128, 256], F32, tag="pk")
                    for e in range(ET):
                        nc.tensor.matmul(
                            pk, wk[:, e, ts(ft, 128)],
                            xt[:, e, SRC[sg]:SRC[sg] + 256],
                            start=(e == 0), stop=(e == ET - 1),
                        )
                    nc.scalar.activation(kt[:, ft, ts(sg, 256)], pk, Ident,
                                         bias=bk_sb[:, ft:ft + 1])
        wk_ctx.__exit__(None, None, None)
        xt_ctx.__exit__(None, None, None)

        # ---- fused scores + attn@v, interleaved per slot -----------------
        # Emission pattern: scores(s3), scores(s2), d(s3), scores(s1),
        # d(s2), scores(s0), d(s1), d(s0) - each d-loop's P tiles and v
        # tiles are ready well before it issues, and scores/d-loop PSUM
        # pools coexist within the 8-bank budget.
        p_pool = top.enter_context(tc.tile_pool(name="pP", bufs=1))
        denr_pool = top.enter_context(tc.tile_pool(name="denr", bufs=1))
        vta_pool = top.enter_context(tc.tile_pool(name="vta", bufs=1))
        v_tiles = []
        for _tt in range(ST):
            v_tiles.append(vta_pool.tile([128, E], F32R, tag=f"vt{_tt}",
                                         name=f"vt_{_tt}"))
        p_tiles = {}
        den_r = {}
        vt_loaded = set()

        with tc.tile_pool(name="mask", bufs=3) as mask_pool, \
             tc.tile_pool(name="osb", bufs=4) as osb_pool, \
             tc.tile_pool(name="dbc", bufs=2) as dbc_pool, \
             tc.tile_pool(name="ps", bufs=4, space="PSUM") as ps_pool, \
             tc.tile_pool(name="pden", bufs=1, space="PSUM") as pden_pool, \
             tc.tile_pool(name="po", bufs=2, space="PSUM") as po_pool, \
             tc.tile_pool(name="pbc", bufs=1, space="PSUM") as pbc_pool:

            def emit_scores(sg):
                T = SLOT_T[sg]
                pden = pden_pool.tile([1, 256], F32, tag="pden",
                                      name=f"pden_{sg}")
                for j in range(T):
                    ps = ps_pool.tile([128, 256], F32, tag="ps",
                                      name=f"ps_{sg}_{j}")
                    for e in range(ET):
                        nc.tensor.matmul(
                            ps, qt[:, e, ts(j, 128)], kt[:, e, ts(sg, 256)],
                            start=(e == 0), stop=(e == ET - 1),
                        )
                    P = p_pool.tile([128, 256], F32R, tag=f"P{j}",
                                    name=f"P_{sg}_{j}", bufs=2)
                    nc.scalar.activation(P, ps, Exp, scale=float(SCALE))
                    m = mask_pool.tile([128, 256], F32R, tag="mask",
                                       name=f"m_{sg}_{j}")
                    nc.sync.dma_start(out=m, in_=masks[MBASE[sg] + j, :, :])
                    nc.vector.tensor_mul(P, P, m)
                    if j not in vt_loaded:
                        vt_loaded.add(j)
                        nc.sync.dma_start(out=v_tiles[j],
                                          in_=v_dram[ts(j, 128), :])
                    nc.tensor.matmul(pden, ones[:, 0:1], P,
                                     start=(j == 0), stop=(j == T - 1))
                    p_tiles[(sg, j)] = P
                dr = denr_pool.tile([1, 256], F32R, tag=f"dr{sg}",
                                    name=f"dr_{sg}")
                with nc.allow_low_precision(reason="f32r storage is fp32"):
                    nc.vector.reciprocal(dr, pden)
                den_r[sg] = dr

            def emit_dloop(sg):
                T = SLOT_T[sg]
                pbc = pbc_pool.tile([128, 256], F32, tag="pbc",
                                    name=f"pbc_{sg}")
                nc.tensor.matmul(pbc, ones[0:1, :], den_r[sg],
                                 start=True, stop=True)
                dbc = dbc_pool.tile([128, 256], F32, tag="dbc",
                                    name=f"dbc_{sg}")
                nc.scalar.copy(dbc, pbc)
                for dt in range(ET):
                    po = po_pool.tile([128, 256], F32, tag="po",
                                      name=f"po_{sg}_{dt}")
                    for j in range(T):
                        nc.tensor.matmul(
                            po, v_tiles[j][:, ts(dt, 128)], p_tiles[(sg, j)],
                            start=(j == 0), stop=(j == T - 1),
                        )
                    osb = osb_pool.tile([128, 256], F32, tag="osb",
                                        name=f"osb_{sg}_{dt}")
                    nc.vector.tensor_mul(osb, po, dbc)
                    nc.sync.dma_start(
                        out=outT[ts(dt, 128), ts(sg, 256)], in_=osb)

            emit_scores(SLOT_ORDER[0])
            for idx, sg in enumerate(SLOT_ORDER):
                nxt = SLOT_ORDER[idx + 1] if idx + 1 < len(SLOT_ORDER) else None
                if nxt is not None:
                    emit_scores(nxt)
                if nxt == SLOT_ORDER[-1]:
                    kt_ctx.__exit__(None, None, None)
                    qt_ctx.__exit__(None, None, None)
                emit_dloop(sg)

    nc.compile()
    return nc


def _get_program():
    if "nc" not in _CACHE:
        _CACHE["nc"] = _build_program()
    return _CACHE["nc"]


def _perm_indices(h):
    return np.concatenate(
        [np.arange(256 * b, 256 * (b + 1)) for b in PERM_BLOCKS[h]])


def _host_prep(x, Wk, bk, Wq, bq, Wv, bv):
    """Build per-core in_maps."""
    f32 = np.float32
    wkT = np.ascontiguousarray(Wk.T.astype(f32))
    wqT = np.ascontiguousarray(Wq.T.astype(f32))
    wvT = np.ascontiguousarray(Wv.T.astype(f32))
    bkp = np.ascontiguousarray(bk.astype(f32).reshape(ET, 128).T)
    bqp = np.ascontiguousarray(bq.astype(f32).reshape(ET, 128).T)
    bvc = np.ascontiguousarray(bv.astype(f32))
    ones128 = np.ones((128, 128), f32)

    in_maps = []
    for c in range(NCORES):
        b, h = divmod(c, 2)
        perm = _perm_indices(h)
        xTb = np.ascontiguousarray(x[b].T.astype(f32)[:, perm])
        m = np.zeros((40, 128, 256), f32)
        for sg in range(4):
            s_g = perm[SRC[sg]:SRC[sg] + 256]
            for j in range(SLOT_T[sg]):
                t_g = perm[128 * j:128 * (j + 1)]
                m[MBASE[sg] + j] = (t_g[:, None] <= s_g[None, :]).astype(f32)
        in_maps.append({
            "xT": xTb, "wkT": wkT, "wqT": wqT, "wvT": wvT,
            "bkp": bkp, "bqp": bqp, "bv": bvc, "masks": m,
            "ones_d": ones128,
        })
    return in_maps


def _assemble(results):
    out = np.empty((B, S, E), np.float32)
    for c in range(NCORES):
        b, h = divmod(c, 2)
        perm = _perm_indices(h)
        oT = results[c]["outT"]  # [E, 1024]
        for sg in range(4):
            rows = perm[SRC[sg]:SRC[sg] + 256]
            out[b, rows, :] = oT[:, 256 * sg:256 * (sg + 1)].T
    return out


def kernel(x, Wk, bk, Wq, bq, Wv, bv):
    _ensure_concourse()
    from concourse.bass_utils import run_bass_kernel_spmd
    nc = _get_program()
    in_maps = _host_prep(x, Wk, bk, Wq, bq, Wv, bv)
    res = run_bass_kernel_spmd(nc, in_maps, list(range(NCORES)))
    return _assemble(res.results)


# revision 34
# speedup vs baseline: 1.0070x; 1.0070x over previous
"""Trainium2 Bass kernel for single-head causal attention (nn_DefaultAttention).

Reference computation (B=4, S=2048, E=1024, fp32):
    k = x @ Wk.T + bk ; q = x @ Wq.T + bq ; v = x @ Wv.T + bv
    sim[b,s,t] = k[b,s]·q[b,t] / sqrt(E), masked to t<=s
    out[b,s]   = softmax_t(sim[b,s,:]) @ v[b,:]
i.e. standard causal attention with Q-role=k, K-role=q, V-role=v.

Sharding: 8 cores = 4 batches x 2 interleaved sequence-quarter sets.
With 256-row query blocks g0..g7 per batch, core h=0 takes {g0,g2,g5,g7}
and h=1 takes {g1,g3,g4,g6} (balanced causal work: 36 quarter-tiles each).
SPMD requires an identical instruction stream on every core, so the
causal structure is made uniform via a host-side per-core column
permutation of x^T plus data-driven 0/1 masks. Score slots sigma=0..3
process T=[16,12,8,4] key-tiles against query blocks at fixed permuted
positions SRC=[1536,1024,512,0]; the permutations place each core's
blocks so every needed key precedes the slot's window:
  h=0 perm (256-blocks): [0,1,2,3,5,4,7,6]
  h=1 perm (256-blocks): [1,0,3,2,4,5,6,7]
Invalid (t>s) positions are zeroed after exp by per-core mask tensors.

All matmuls run as float32r (single-pass fp32, 1 cycle/row at N>=256).
Projections compute kT/qT in [feature, seq] layout directly (host feeds
x^T and W^T so no on-chip transposes); scores are computed transposed
(simT[t,s]) so P=exp(simT) feeds the attn@v matmul as the streaming
operand with v slices stationary; output comes out as out^T[d,s] and is
transposed back on the host. Softmax denominator = ones-vector matmul
over P, normalization by a broadcast reciprocal at the end.
"""

import numpy as np


def _ensure_concourse():
    try:
        import concourse  # noqa: F401
    except ImportError:
        import sys
        for p in ("/opt/trn_rl_repo", "/root/.axon_site/_ro/trn_rl_repo"):
            if p not in sys.path:
                sys.path.append(p)
        import concourse  # noqa: F401


E = 1024
S = 2048
B = 4
NCORES = 8
ET = E // 128    # 8 feature tiles
ST = S // 128    # 16 key tiles
SCALE = 1.0 / np.sqrt(np.float32(E))
SLOT_T = (16, 12, 8, 4)        # key-128-tiles per score slot (uniform)
SRC = (1536, 1024, 512, 0)     # permuted query-col base per slot
MBASE = (0, 16, 28, 36)        # flat mask index base per slot
SLOT_ORDER = (3, 2, 1, 0)      # processing order (smallest T first)
PERM_BLOCKS = {0: [0, 1, 2, 3, 5, 4, 7, 6], 1: [1, 0, 3, 2, 4, 5, 6, 7]}

_CACHE = {}


def _build_program():
    _ensure_concourse()
    from contextlib import ExitStack
    import concourse.tile as tile
    import concourse.bass as bass
    from concourse import bacc, mybir

    F32 = mybir.dt.float32
    F32R = mybir.dt.float32r
    ts = bass.ts
    Exp = mybir.ActivationFunctionType.Exp
    Ident = mybir.ActivationFunctionType.Identity

    nc = bacc.Bacc("TRN2", target_bir_lowering=False, debug=False)

    xT = nc.dram_tensor("xT", [E, S], F32R, kind="ExternalInput").ap()
    wkT = nc.dram_tensor("wkT", [E, E], F32R, kind="ExternalInput").ap()
    wqT = nc.dram_tensor("wqT", [E, E], F32R, kind="ExternalInput").ap()
    wvT = nc.dram_tensor("wvT", [E, E], F32R, kind="ExternalInput").ap()
    bkp = nc.dram_tensor("bkp", [128, ET], F32, kind="ExternalInput").ap()
    bqp = nc.dram_tensor("bqp", [128, ET], F32, kind="ExternalInput").ap()
    bv = nc.dram_tensor("bv", [E], F32, kind="ExternalInput").ap()
    masks = nc.dram_tensor("masks", [40, 128, 256], F32R, kind="ExternalInput").ap()
    ones_d = nc.dram_tensor("ones_d", [128, 128], F32R, kind="ExternalInput").ap()
    outT = nc.dram_tensor("outT", [E, 1024], F32, kind="ExternalOutput").ap()
    v_dram = nc.dram_tensor("v_spill", [S, E], F32R).ap()

    with tile.TileContext(nc) as tc, ExitStack() as top:
        # ---- persistent smalls -------------------------------------------
        smalls = top.enter_context(tc.tile_pool(name="smalls", bufs=1))
        ones = smalls.tile([128, 128], F32R)
        nc.sync.dma_start(out=ones, in_=ones_d)
        bk_sb = smalls.tile([128, ET], F32)
        nc.sync.dma_start(out=bk_sb, in_=bkp)
        bq_sb = smalls.tile([128, ET], F32)
        nc.sync.dma_start(out=bq_sb, in_=bqp)

        # Warm the ACT function tables (Identity/Exp/Copy) up front so the
        # LoadActFuncSet DMA doesn't queue behind the bulk loads later.
        scratch = smalls.tile([1, 8], F32)
        nc.vector.memset(scratch, 0.0)
        nc.scalar.activation(scratch, scratch, Ident, bias=0.0, scale=1.0)
        nc.scalar.activation(scratch, scratch, Exp, scale=1.0)
        nc.scalar.copy(scratch, scratch)

        # x^T resident for the three projection phases (closed after K)
        xt_ctx = tc.tile_pool(name="xt", bufs=1)
        xt_pool = xt_ctx.__enter__()
        xt = xt_pool.tile([128, ET, S], F32R)

        def load_xt(cb):
            for e in range(ET):
                nc.sync.dma_start(
                    out=xt[:, e, ts(cb, 512)],
                    in_=xT.rearrange("(e p) s -> p e s", p=128)[:, e, ts(cb, 512)],
                )

        # Weight pools opened early; close LIFO (wv after V, wq after Q,
        # wk after K). The wk/wq loads are trickled into the spill stream.
        wk_ctx = tc.tile_pool(name="wk", bufs=1)
        wk_pool = wk_ctx.__enter__()
        wk = wk_pool.tile([128, ET, E], F32R)
        wq_ctx = tc.tile_pool(name="wq", bufs=1)
        wq_pool = wq_ctx.__enter__()
        wq = wq_pool.tile([128, ET, E], F32R)

        wkq_thunks = []
        for _db in range(2):
            for _e in range(ET):
                wkq_thunks.append((wq, wqT, _e, _db))
        for _db in range(2):
            for _e in range(ET):
                wkq_thunks.append((wk, wkT, _e, _db))
        # V-phase DMA is budget-limited (loads + v spills saturate ~360GB/s
        # for the whole phase), so only wq-db0 is trickled there; the rest
        # trickles during Q, whose DMA queue is otherwise idle. wq db1 is
        # first consumed at ft=4, a third into Q; wk only at phase K.

        def emit_wkq(n):
            for _ in range(n):
                if not wkq_thunks:
                    return
                dst, src, _e, _db = wkq_thunks.pop(0)
                nc.sync.dma_start(
                    out=dst[:, _e, ts(_db, 512)],
                    in_=src.rearrange("(e p) f -> p e f", p=128)[:, _e, ts(_db, 512)],
                )

        # ---- phase V: v = x @ Wv.T + bv  (natural [t, d]), spilled to DRAM
        with tc.tile_pool(name="wv", bufs=1) as wv_pool, \
             tc.tile_pool(name="vsb", bufs=16) as vsb_pool, \
             tc.tile_pool(name="bvb", bufs=1) as bvb_pool, \
             tc.tile_pool(name="pv", bufs=6, space="PSUM") as pv_pool:
            wv = wv_pool.tile([128, ET, E], F32R)

            def load_wv(db):
                for e in range(ET):
                    nc.sync.dma_start(
                        out=wv[:, e, ts(db, 512)],
                        in_=wvT.rearrange("(e p) d -> p e d", p=128)[:, e, ts(db, 512)],
                    )
            bvb = bvb_pool.tile([128, E], F32)
            bv_bcast = bass.AP(tensor=bv.tensor, offset=bv.offset,
                               ap=[[0, 128]] + list(bv.ap))
            def load_wv_half(db, half):
                for e in range(4 * half, 4 * half + 4):
                    nc.sync.dma_start(
                        out=wv[:, e, ts(db, 512)],
                        in_=wvT.rearrange("(e p) d -> p e d", p=128)[:, e, ts(db, 512)],
                    )
            load_wv(0)
            load_xt(0)
            nc.sync.dma_start(out=bvb, in_=bv_bcast)
            load_xt(1)
            load_xt(2)
            load_wv_half(1, 0)
            load_xt(3)
            load_wv_half(1, 1)

            for db in range(2):
                for tt in range(ST):
                    pv = pv_pool.tile([128, 512], F32, tag="pv")
                    for e in range(ET):
                        nc.tensor.matmul(
                            pv, xt[:, e, ts(tt, 128)], wv[:, e, ts(db, 512)],
                            start=(e == 0), stop=(e == ET - 1),
                        )
                    vhalf = vsb_pool.tile([128, 512], F32R, tag="vsb")
                    nc.vector.tensor_add(vhalf, pv, bvb[:, ts(db, 512)])
                    nc.sync.dma_start(
                        out=v_dram[ts(tt, 128), ts(db, 512)], in_=vhalf)
                    if db == 0 and tt < 8:
                        emit_wkq(1)

        # ---- phase Q: qT = (x @ Wq.T + bq)^T  ([f, t], all 2048 t) -------
        qt_ctx = tc.tile_pool(name="qt", bufs=1, side="right")
        qt_pool = qt_ctx.__enter__()
        qt = qt_pool.tile([128, ET, S], F32R)
        with tc.tile_pool(name="pq", bufs=8, space="PSUM") as pq_pool:
            for ft in range(ET):
                for sb4 in range(4):
                    pq = pq_pool.tile([128, 512], F32, tag="pq")
                    for e in range(ET):
                        nc.tensor.matmul(
                            pq, wq[:, e, ts(ft, 128)], xt[:, e, ts(sb4, 512)],
                            start=(e == 0), stop=(e == ET - 1),
                        )
                    nc.scalar.activation(qt[:, ft, ts(sb4, 512)], pq, Ident,
                                         bias=bq_sb[:, ft:ft + 1])
                    emit_wkq(1)
        emit_wkq(99)
        wq_ctx.__exit__(None, None, None)

        # ---- phase K: kT for local queries, slot sg at cols [256*sg,..) --
        kt_ctx = tc.tile_pool(name="kt", bufs=1, side="right")
        kt_pool = kt_ctx.__enter__()
        kt = kt_pool.tile([128, ET, 1024], F32R)
        with tc.tile_pool(name="pk", bufs=6, space="PSUM") as pk_pool:
            for ft in range(ET):
                for sg in SLOT_ORDER:
                    pk = pk_pool.tile([128, 256], F32, tag="pk")
                    for e in range(ET):
                        nc.tensor.matmul(
                            pk, wk[:, e, ts(ft, 128)],
                            xt[:, e, SRC[sg]:SRC[sg] + 256],
                            start=(e == 0), stop=(e == ET - 1),
                        )
                    nc.scalar.activation(kt[:, ft, ts(sg, 256)], pk, Ident,
                                         bias=bk_sb[:, ft:ft + 1])
        wk_ctx.__exit__(None, None, None)
        xt_ctx.__exit__(None, None, None)

        # ---- fused scores + attn@v, interleaved per slot -----------------
        # Emission pattern: scores(s3), scores(s2), d(s3), scores(s1),
        # d(s2), scores(s0), d(s1), d(s0) - each d-loop's P tiles and v
        # tiles are ready well before it issues, and scores/d-loop PSUM
        # pools coexist within the 8-bank budget.
        p_pool = top.enter_context(tc.tile_pool(name="pP", bufs=1))
        denr_pool = top.enter_context(tc.tile_pool(name="denr", bufs=1))
        vta_pool = top.enter_context(tc.tile_pool(name="vta", bufs=1))
        v_tiles = []
        for _tt in range(ST):
            v_tiles.append(vta_pool.tile([128, E], F32R, tag=f"vt{_tt}",
                                         name=f"vt_{_tt}"))
        p_tiles = {}
        den_r = {}
        vt_loaded = set()

        with tc.tile_pool(name="mask", bufs=4) as mask_pool, \
             tc.tile_pool(name="osb", bufs=4) as osb_pool, \
             tc.tile_pool(name="dbc", bufs=2) as dbc_pool, \
             tc.tile_pool(name="ps", bufs=4, space="PSUM") as ps_pool, \
             tc.tile_pool(name="pden", bufs=1, space="PSUM") as pden_pool, \
             tc.tile_pool(name="po", bufs=2, space="PSUM") as po_pool, \
             tc.tile_pool(name="pbc", bufs=1, space="PSUM") as pbc_pool:

            def emit_scores(sg):
                T = SLOT_T[sg]
                pden = pden_pool.tile([1, 256], F32, tag="pden",
                                      name=f"pden_{sg}")
                for j in range(T):
                    ps = ps_pool.tile([128, 256], F32, tag="ps",
                                      name=f"ps_{sg}_{j}")
                    for e in range(ET):
                        nc.tensor.matmul(
                            ps, qt[:, e, ts(j, 128)], kt[:, e, ts(sg, 256)],
                            start=(e == 0), stop=(e == ET - 1),
                        )
                    P = p_pool.tile([128, 256], F32R, tag=f"P{j}",
                                    name=f"P_{sg}_{j}", bufs=2)
                    nc.scalar.activation(P, ps, Exp, scale=float(SCALE))
                    m = mask_pool.tile([128, 256], F32R, tag="mask",
                                       name=f"m_{sg}_{j}")
                    nc.sync.dma_start(out=m, in_=masks[MBASE[sg] + j, :, :])
                    nc.vector.tensor_mul(P, P, m)
                    if j not in vt_loaded:
                        vt_loaded.add(j)
                        nc.sync.dma_start(out=v_tiles[j],
                                          in_=v_dram[ts(j, 128), :])
                    nc.tensor.matmul(pden, ones[:, 0:1], P,
                                     start=(j == 0), stop=(j == T - 1))
                    p_tiles[(sg, j)] = P
                dr = denr_pool.tile([1, 256], F32R, tag=f"dr{sg}",
                                    name=f"dr_{sg}")
                with nc.allow_low_precision(reason="f32r storage is fp32"):
                    nc.vector.reciprocal(dr, pden)
                den_r[sg] = dr

            def emit_dloop(sg):
                T = SLOT_T[sg]
                pbc = pbc_pool.tile([128, 256], F32, tag="pbc",
                                    name=f"pbc_{sg}")
                nc.tensor.matmul(pbc, ones[0:1, :], den_r[sg],
                                 start=True, stop=True)
                dbc = dbc_pool.tile([128, 256], F32, tag="dbc",
                                    name=f"dbc_{sg}")
                nc.scalar.copy(dbc, pbc)
                for dt in range(ET):
                    po = po_pool.tile([128, 256], F32, tag="po",
                                      name=f"po_{sg}_{dt}")
                    for j in range(T):
                        nc.tensor.matmul(
                            po, v_tiles[j][:, ts(dt, 128)], p_tiles[(sg, j)],
                            start=(j == 0), stop=(j == T - 1),
                        )
                    osb = osb_pool.tile([128, 256], F32, tag="osb",
                                        name=f"osb_{sg}_{dt}")
                    nc.vector.tensor_mul(osb, po, dbc)
                    nc.sync.dma_start(
                        out=outT[ts(dt, 128), ts(sg, 256)], in_=osb)

            emit_scores(SLOT_ORDER[0])
            for idx, sg in enumerate(SLOT_ORDER):
                nxt = SLOT_ORDER[idx + 1] if idx + 1 < len(SLOT_ORDER) else None
                if nxt is not None:
                    emit_scores(nxt)
                if nxt == SLOT_ORDER[-1]:
                    kt_ctx.__exit__(None, None, None)
                    qt_ctx.__exit__(None, None, None)
                emit_dloop(sg)

    nc.compile()
    return nc


def _get_program():
    if "nc" not in _CACHE:
        _CACHE["nc"] = _build_program()
    return _CACHE["nc"]


def _perm_indices(h):
    return np.concatenate(
        [np.arange(256 * b, 256 * (b + 1)) for b in PERM_BLOCKS[h]])


def _host_prep(x, Wk, bk, Wq, bq, Wv, bv):
    """Build per-core in_maps."""
    f32 = np.float32
    wkT = np.ascontiguousarray(Wk.T.astype(f32))
    wqT = np.ascontiguousarray(Wq.T.astype(f32))
    wvT = np.ascontiguousarray(Wv.T.astype(f32))
    bkp = np.ascontiguousarray(bk.astype(f32).reshape(ET, 128).T)
    bqp = np.ascontiguousarray(bq.astype(f32).reshape(ET, 128).T)
    bvc = np.ascontiguousarray(bv.astype(f32))
    ones128 = np.ones((128, 128), f32)

    in_maps = []
    for c in range(NCORES):
        b, h = divmod(c, 2)
        perm = _perm_indices(h)
        xTb = np.ascontiguousarray(x[b].T.astype(f32)[:, perm])
        m = np.zeros((40, 128, 256), f32)
        for sg in range(4):
            s_g = perm[SRC[sg]:SRC[sg] + 256]
            for j in range(SLOT_T[sg]):
                t_g = perm[128 * j:128 * (j + 1)]
                m[MBASE[sg] + j] = (t_g[:, None] <= s_g[None, :]).astype(f32)
        in_maps.append({
            "xT": xTb, "wkT": wkT, "wqT": wqT, "wvT": wvT,
            "bkp": bkp, "bqp": bqp, "bv": bvc, "masks": m,
            "ones_d": ones128,
        })
    return in_maps


def _assemble(results):
    out = np.empty((B, S, E), np.float32)
    for c in range(NCORES):
        b, h = divmod(c, 2)
        perm = _perm_indices(h)
        oT = results[c]["outT"]  # [E, 1024]
        for sg in range(4):
            rows = perm[SRC[sg]:SRC[sg] + 256]
            out[b, rows, :] = oT[:, 256 * sg:256 * (sg + 1)].T
    return out


def kernel(x, Wk, bk, Wq, bq, Wv, bv):
    _ensure_concourse()
    from concourse.bass_utils import run_bass_kernel_spmd
    nc = _get_program()
    in_maps = _host_prep(x, Wk, bk, Wq, bq, Wv, bv)
    res = run_bass_kernel_spmd(nc, in_maps, list(range(NCORES)))
    return _assemble(res.results)
